# revision 14
# baseline (speedup 1.0000x reference)
"""Trainium2 Bass kernel for CrossCAM: cross channel-attention + 1x1 conv.

Reference computation (per batch b, C=64, N=H*W=16384):
    E_t = t_v @ t_v.T                     [C, C]   (t_v = template[b] as [C, N])
    E_r = r_v @ r_v.T
    attn_x = softmax(rowmax(E_x) - E_x)   rows; == exp(rowmin-E)/sum(exp(rowmin-E))
    t_out = gamma * (r_attn @ t_v) + t_v
    r_out = omega * (t_attn @ r_v) + r_v
    out   = conv_w @ concat(t_out, r_out) + conv_b        [64, N]

Key algebraic restructuring: the 1x1 conv distributes over the residual, so
    out = M_t @ t_v + M_r @ r_v + conv_b
    M_t = gamma * (w1 @ r_attn) + w1,   M_r = omega * (w2 @ t_attn) + w2
with w1 = conv_w[:, :64], w2 = conv_w[:, 64:].  Only ONE streaming pass over
the big tensors is needed; everything attention-related is 64x64.

Data layout on device ("split" layout): each [64, 16384] map is held in SBUF
as [128, 8192]: partition p = h*64+c holds t_v[c, h*8192:(h+1)*8192].  The
final matmul then runs with full K=128 using block-diagonal weights
W_x = blockdiag(M_xT, M_xT) [128, 128], and out128 in the same split layout
is contiguous-compatible with the HBM output tensor.

Sharding: pure data parallel, 2 batches per core on 8 cores.

When gamma == omega == 0 (the spec's input fill), M_t = w1 and M_r = w2 are
input constants: the attention pipeline is mathematically irrelevant (it is
multiplied by zero), so a fast program that skips it is exact.  The general
program computes the full attention path on device.
"""

import numpy as np

import concourse.tile as tile
from concourse import bacc, mybir
from concourse import bass_utils
from concourse.vector_clock import ScopedClock

F32_placeholder = None  # keep line numbers sane


class _LightEndTileContext(tile.TileContext):
    """Stock Tile ending except the post-clear all-engine barrier uses the
    sem-only variant (no per-engine InstDrain).  The reset protocol -- sync
    drain carrying the global-clock waits, a FULL butterfly before the
    dma_reset+sem_clear -- is unchanged; only the final barrier, which for a
    top-level context is immediately followed by the NEFF wrapper's own
    arrival chain (itself a barrier), is lightened."""

    def _drain_and_barrier(self, tick_clock, wait_clock):
        drain_inst = self.nc.sync.drain()
        wait_clock.add_sem_waits(
            drain_inst.ins, ScopedClock({None: tick_clock.global_clock})
        )
        self.nc.all_engine_barrier()
        assert self.sems is not None
        popped = self.nc._tile_sem_poison_stack.pop()
        assert popped is self._sem_poison
        self.nc.clear_and_free_semaphores(list(self.sems.allocated().values()))
        self.nc.all_engine_barrier(sem_only=True)

from concourse.vector_clock import ScopedClock

F32 = mybir.dt.float32


class _FastEndTileContext(tile.TileContext):
    """TileContext with a minimal kernel ending.

    Stock Tile ends every kernel with: sync drain (waits the full global
    clock) -> all-engine EVSEM butterfly -> semaphore clears -> second
    butterfly.  On HW the two butterflies cost ~6-7us of pure sequencer
    latency AFTER the last DMA byte has landed, all inside the profiled
    exec window.  For a top-level context we only need: (1) every sem
    user done before the clears, (2) sems back to 0 for re-execution.
    Both are satisfied by attaching the global-clock sem waits to a
    gpsimd drain and clearing from gpsimd right after; other engines
    simply end their streams.
    """

    def _drain_and_barrier(self, tick_clock, wait_clock):
        drain_inst = self.nc.gpsimd.drain()
        wait_clock.add_sem_waits(
            drain_inst.ins, ScopedClock({None: tick_clock.global_clock})
        )
        assert self.sems is not None
        popped = self.nc._tile_sem_poison_stack.pop()
        assert popped is self._sem_poison
        self.nc.clear_and_free_semaphores(list(self.sems.allocated().values()))

B, C, H, W = 16, 64, 128, 128
N = H * W          # 16384
NCORES = 8
BPC = B // NCORES  # batches per core
HALF = N // 2      # 8192
CK = 512           # matmul free-dim chunk
NCHUNK = HALF // CK  # 16

_programs: dict[tuple, object] = {}

# DMA engine knobs (A/B-tested on hardware).  Only sync ("SP") and scalar
# ("Activation") have HWDGE rings; gpsimd is the slow SWDGE path.  Loads
# stream on sync, bulk stores on scalar; the last batch's tail stores move
# to sync (idle by then) so the final store pays no DGE spin-up.
LOAD_ENGINE = "sync"
STORE_ENGINE = "scalar"
# PE dtype for the big streaming matmuls: "f32" (exact, LOW/HIGH double
# pass, 55us of PE per core) or "f32r" (relaxed fp32, single pass at
# free-dim >= 256, ~45us).  f32r errs ~1.6e-4 rel here -- way inside the
# 2e-2 gate -- and keeps the PE off the critical path.
MM_DTYPE = "f32r"
# Store chunk width in CK units for the attention-path store groups.
OC_WIDE = 2
# Fast path: per-map load segments in columns.  Hardware-measured DMA
# descriptor rates (one descriptor per SBUF partition row): 8KB rows
# ~25.8 GB/s/engine (best), 4KB ~23, 16KB ~24 (worse than 8KB!), so the
# bulk uses 2048-col (8KB-row) tiles.  The small final segments shorten
# the dependency chain that runs after the last load lands
# (matmul -> PSUM drain -> store), which is fully exposed in exec time.
# 256 is the f32r floor (free-dim >= 256 keeps the PE at 1 cycle/row).
SEGS = (2048, 2048, 2048, 1024, 512, 512)
assert sum(SEGS) == HALF


def _build_program(with_attn: bool):
    nc = bacc.Bacc(
        "TRN2",
        target_bir_lowering=False,
        debug=False,
        enable_asserts=False,
        num_devices=NCORES,
    )
    # float32r = same 4-byte fp32 bits, but the PE runs 1 cycle/row (vs 4
    # for strict fp32) at free-dim >= 256, with relaxed internal rounding.
    # The whole produce-consume chain must carry the dtype.
    MMDT = (
        mybir.dt.float32r if (MM_DTYPE == "f32r" and not with_attn) else F32
    )
    t_in = nc.dram_tensor("t_in", [BPC, C, N], MMDT, kind="ExternalInput").ap()
    r_in = nc.dram_tensor("r_in", [BPC, C, N], MMDT, kind="ExternalInput").ap()
    wt0 = nc.dram_tensor("wt0", [128, 128], MMDT, kind="ExternalInput").ap()
    wr0 = nc.dram_tensor("wr0", [128, 128], MMDT, kind="ExternalInput").ap()
    bias2 = nc.dram_tensor("bias2", [128, 1], F32, kind="ExternalInput").ap()
    if with_attn:
        cwt1_d = nc.dram_tensor("cwt1", [C, C], F32, kind="ExternalInput").ap()
        cwt2_d = nc.dram_tensor("cwt2", [C, C], F32, kind="ExternalInput").ap()
        gam_d = nc.dram_tensor("gam2", [128, 1], F32, kind="ExternalInput").ap()
        omg_d = nc.dram_tensor("omg2", [128, 1], F32, kind="ExternalInput").ap()
        ident_d = nc.dram_tensor("ident", [128, 128], F32, kind="ExternalInput").ap()
    out = nc.dram_tensor("out", [BPC, C, N], F32, kind="ExternalOutput").ap()

    Exp = mybir.ActivationFunctionType.Exp
    Ident = mybir.ActivationFunctionType.Identity

    with _LightEndTileContext(nc) as tc:
        from contextlib import ExitStack

        with ExitStack() as ctx:
            const = ctx.enter_context(tc.tile_pool(name="const", bufs=1))
            vpool = ctx.enter_context(tc.tile_pool(name="v", bufs=2))
            pspool = ctx.enter_context(
                tc.tile_pool(name="ps", bufs=8 if not with_attn else 4, space="PSUM")
            )
            ocpool = ctx.enter_context(tc.tile_pool(name="oc", bufs=4))
            if with_attn:
                tppool = ctx.enter_context(tc.tile_pool(name="tp", bufs=2, space="PSUM"))
                egpool = ctx.enter_context(tc.tile_pool(name="eg", bufs=1, space="PSUM"))
                p1pool = ctx.enter_context(tc.tile_pool(name="p1", bufs=1, space="PSUM"))
                atpool = ctx.enter_context(tc.tile_pool(name="at", bufs=3))
                smpool = ctx.enter_context(tc.tile_pool(name="sm", bufs=2))

            cld = nc.scalar if not with_attn else nc.sync
            Wt = const.tile([128, 128], MMDT, tag="Wt")
            cld.dma_start(Wt[:], wt0[:])
            Wr = const.tile([128, 128], MMDT, tag="Wr")
            cld.dma_start(Wr[:], wr0[:])
            bias_sb = const.tile([128, 1], F32, tag="bias")
            cld.dma_start(bias_sb[:], bias2[:])
            if with_attn:
                cwt1 = const.tile([C, C], F32, tag="cwt1")
                nc.sync.dma_start(cwt1[:], cwt1_d[:])
                cwt2 = const.tile([C, C], F32, tag="cwt2")
                nc.sync.dma_start(cwt2[:], cwt2_d[:])
                gam = const.tile([128, 1], F32, tag="gam")
                nc.sync.dma_start(gam[:], gam_d[:])
                omg = const.tile([128, 1], F32, tag="omg")
                nc.sync.dma_start(omg[:], omg_d[:])
                ident = const.tile([128, 128], F32, tag="ident")
                nc.sync.dma_start(ident[:], ident_d[:])

            for i in range(BPC):
                ld = getattr(nc, LOAD_ENGINE if LOAD_ENGINE != "alt" else "sync")
                if with_attn:
                    # block-split layout: partition h*64+c <- v[c, h*HALF+n]
                    t128 = vpool.tile([128, HALF], MMDT, tag="t")
                    r128 = vpool.tile([128, HALF], MMDT, tag="r")
                    ld.dma_start(t128[0:64, :], t_in[i, :, 0:HALF])
                    ld.dma_start(t128[64:128, :], t_in[i, :, HALF:N])
                    ld.dma_start(r128[0:64, :], r_in[i, :, 0:HALF])
                    ld.dma_start(r128[64:128, :], r_in[i, :, HALF:N])
                else:
                    # interleaved layout: partition 2c+h <- v[c, h*HALF+n].
                    # One DMA covers all 128 partitions -> all 16 SBUF AXI
                    # ports engage concurrently (the split form above only
                    # drives half the ports per transfer).  Each map is
                    # loaded as SEGS column segments (t/r interleaved) so
                    # the first matmuls start as soon as segment 0 lands.
                    t_il = t_in[i].rearrange("c (h n) -> (c h) n", h=2)
                    r_il = r_in[i].rearrange("c (h n) -> (c h) n", h=2)
                    tq, rq = [], []
                    o = 0
                    for q, w in enumerate(SEGS):
                        tt = vpool.tile([128, w], MMDT, tag=f"t{q}")
                        ld.dma_start(tt[:], t_il[:, o : o + w])
                        tq.append((o, tt))
                        rr = vpool.tile([128, w], MMDT, tag=f"r{q}")
                        ld.dma_start(rr[:], r_il[:, o : o + w])
                        rq.append((o, rr))
                        o += w

                if with_attn:
                    attn = {}
                    for name, v128 in (("t", t128), ("r", r128)):
                        # E_grand[a, b] = sum_f v128[a, f] v128[b, f], via
                        # PE-transposed chunks; E = diag-fold of E_grand.
                        eg_ps = egpool.tile([128, 128], F32, tag="eg")
                        for g in range(HALF // CK):
                            tp = tppool.tile([128, CK], F32, tag="tp")
                            for q in range(4):
                                k = 4 * g + q
                                nc.tensor.transpose(
                                    tp[:, 128 * q : 128 * (q + 1)],
                                    v128[:, 128 * k : 128 * (k + 1)],
                                    ident[:],
                                )
                            at = atpool.tile([128, CK], F32, tag="at")
                            nc.scalar.copy(at[:], tp[:])
                            for q in range(4):
                                k = 4 * g + q
                                sl = at[:, 128 * q : 128 * (q + 1)]
                                nc.tensor.matmul(
                                    eg_ps[:],
                                    sl,
                                    sl,
                                    start=(k == 0),
                                    stop=(k == HALF // 128 - 1),
                                )
                        egs = smpool.tile([128, 128], F32, tag="egs")
                        nc.vector.tensor_copy(egs[:], eg_ps[:])
                        eglow = smpool.tile([C, C], F32, tag="eglow")
                        nc.sync.dma_start(eglow[:], egs[64:128, 64:128])
                        e = smpool.tile([C, C], F32, tag="e")
                        nc.vector.tensor_add(e[:], egs[0:64, 0:64], eglow[:])
                        # softmax(rowmax(E)-E) == exp(rowmin(E)-E)/sum(...)
                        rmin = smpool.tile([C, 1], F32, tag="rmin")
                        nc.vector.tensor_reduce(
                            rmin[:], e[:], axis=mybir.AxisListType.X,
                            op=mybir.AluOpType.min,
                        )
                        p = smpool.tile([C, C], F32, tag="p")
                        rsum = smpool.tile([C, 1], F32, tag="rsum")
                        nc.scalar.activation(
                            p[:], e[:], Exp, bias=rmin[:], scale=-1.0,
                            accum_out=rsum[:],
                        )
                        rinv = smpool.tile([C, 1], F32, tag="rinv")
                        nc.vector.reciprocal(rinv[:], rsum[:])
                        a = smpool.tile([C, C], F32, tag=f"attn_{name}")
                        nc.vector.tensor_scalar_mul(a[:], p[:], rinv[:])
                        attn[name] = a

                    # W_x diag blocks: M_tT = gamma*(w1@r_attn).T + w1T, etc.
                    # (w1@r_attn).T = r_attn.T.T @ w1T = matmul(lhsT=r_attn, rhs=w1T)
                    for wtile, a, cw, g_ap in (
                        (Wt, attn["r"], cwt1, gam),
                        (Wr, attn["t"], cwt2, omg),
                    ):
                        p1 = p1pool.tile([C, C], F32, tag="p1")
                        nc.tensor.matmul(p1[:], a[:], cw[:], start=True, stop=True)
                        tmp = smpool.tile([C, C], F32, tag="tmp")
                        nc.vector.tensor_scalar_mul(tmp[:], p1[:], g_ap[0:64, :])
                        nc.vector.tensor_add(wtile[0:64, 0:64], tmp[:], cw[:])
                        nc.sync.dma_start(wtile[64:128, 64:128], wtile[0:64, 0:64])

                # out128 = Wt.T @ t128 + Wr.T @ r128 + bias (same layout as v)
                st = getattr(nc, STORE_ENGINE)
                out_il = None
                if not with_attn:
                    out_il = out[i].rearrange("c (h n) -> (c h) n", h=2)

                def _col_slice(tiles, o, w=CK):
                    for so, tt in reversed(tiles):
                        if o >= so:
                            return tt[:, o - so : o - so + w]
                    raise AssertionError(o)

                def _seg_slice(tiles, j):
                    return _col_slice(tiles, CK * j)

                def t_chunk(j):
                    if with_attn:
                        return t128[:, CK * j : CK * (j + 1)]
                    return _seg_slice(tq, j)

                def r_chunk(j):
                    if with_attn:
                        return r128[:, CK * j : CK * (j + 1)]
                    return _seg_slice(rq, j)

                def emit_group(j0, n, wide, act_engines=None, store_eng=None):
                    """n chunks from j0; PSUM drained to oc tiles of
                    `wide` chunks, each stored with one DMA."""
                    sg = st if store_eng is None else store_eng
                    pss = []
                    for q in range(n):
                        j = j0 + q
                        ps = pspool.tile([128, CK], F32, tag="ps")
                        nc.tensor.matmul(
                            ps[:], Wt[:], t_chunk(j),
                            start=True, stop=False,
                        )
                        pss.append((j, ps))
                    for j, ps in pss:
                        nc.tensor.matmul(
                            ps[:], Wr[:], r_chunk(j),
                            start=False, stop=True,
                        )
                    oc = None
                    for idx, (j, ps) in enumerate(pss):
                        w = idx % wide
                        if w == 0:
                            oc = ocpool.tile(
                                [128, CK * wide], F32, tag=f"oc{wide}"
                            )
                        eng = nc.scalar if act_engines is None else act_engines[idx]
                        if eng is nc.scalar:
                            eng.activation(
                                oc[:, CK * w : CK * (w + 1)], ps[:],
                                Ident, bias=bias_sb[:], scale=1.0,
                            )
                        else:
                            eng.tensor_scalar_add(
                                oc[:, CK * w : CK * (w + 1)], ps[:], bias_sb[:]
                            )
                        if w < wide - 1:
                            continue
                        jw0 = j - (wide - 1)
                        span = CK * wide
                        if with_attn:
                            sg.dma_start(
                                out[i, :, CK * jw0 : CK * jw0 + span],
                                oc[0:64, :],
                            )
                            sg.dma_start(
                                out[i, :, HALF + CK * jw0 : HALF + CK * jw0 + span],
                                oc[64:128, :],
                            )
                        else:
                            sg.dma_start(
                                out_il[:, CK * jw0 : CK * jw0 + span], oc[:]
                            )

                def emit_tail_piece(col0, w, act_eng, store_eng):
                    """Tail piece [col0, col0+w): matmul -> PSUM drain ->
                    store, engines chosen so consecutive pieces overlap.
                    w >= 256 keeps f32r at 1 cycle/row.  Full-width tiles
                    sliced to w — no extra PSUM/SBUF tags."""
                    ps = pspool.tile([128, CK], F32, tag="ps")
                    nc.tensor.matmul(
                        ps[:, 0:w], Wt[:], _col_slice(tq, col0, w),
                        start=True, stop=False,
                    )
                    nc.tensor.matmul(
                        ps[:, 0:w], Wr[:], _col_slice(rq, col0, w),
                        start=False, stop=True,
                    )
                    oc = ocpool.tile([128, CK], F32, tag="oc1")
                    if act_eng is nc.scalar:
                        act_eng.activation(
                            oc[:, 0:w], ps[:, 0:w], Ident,
                            bias=bias_sb[:], scale=1.0,
                        )
                    else:
                        act_eng.tensor_scalar_add(
                            oc[:, 0:w], ps[:, 0:w], bias_sb[:]
                        )
                    store_eng.dma_start(
                        out_il[:, col0 : col0 + w], oc[:, 0:w]
                    )

                if with_attn:
                    for g in range(NCHUNK // 4):
                        emit_group(4 * g, 4, OC_WIDE)
                else:
                    # Bulk: groups of 4 chunks, [128, 2048] oc tiles, 8KB-row
                    # stores on scalar.  Tail: chunk 15 is split into two
                    # 256-col pieces matching the two final 256-col load
                    # segments, so the post-last-load chain is mm(256) ->
                    # drain(256) -> store(128KB).  For the LAST batch the
                    # scalar ring is still clogged with the (12,13) group
                    # store while loads occupy the SDMA engines, so the
                    # final pieces ride the sync ring (idle once loads are
                    # triggered) and chunk 14 moves to gpsimd's SWDGE ring
                    # (parallel, off the critical chain).
                    emit_group(0, 4, 4)
                    emit_group(4, 4, 4)
                    emit_group(8, 4, 4)
                    emit_group(12, 2, 2)
                    emit_tail_piece(CK * 14, CK, nc.scalar, st)
                    emit_tail_piece(CK * 15, CK, nc.vector, nc.sync)

    nc.compile()
    return nc


def _get_program(with_attn: bool):
    key = (with_attn, LOAD_ENGINE, STORE_ENGINE, MM_DTYPE, OC_WIDE)
    prog = _programs.get(key)
    if prog is None:
        prog = _build_program(with_attn)
        _programs[key] = prog
    return prog


def make_in_maps(template_map, roi_map, gamma, omega, conv_w, conv_b):
    """Host-side prep: per-core input dicts + which program variant to use."""
    template_map = np.ascontiguousarray(np.asarray(template_map, dtype=np.float32))
    roi_map = np.ascontiguousarray(np.asarray(roi_map, dtype=np.float32))
    conv_w = np.asarray(conv_w, dtype=np.float32)
    conv_b = np.asarray(conv_b, dtype=np.float32)
    g = float(np.asarray(gamma).reshape(-1)[0])
    o = float(np.asarray(omega).reshape(-1)[0])
    with_attn = not (g == 0.0 and o == 0.0)

    w1T = np.ascontiguousarray(conv_w[:, :C].T)  # [c, o]
    w2T = np.ascontiguousarray(conv_w[:, C:].T)
    if with_attn:
        # block-split layout: W[h*64+c, h*64+o] = wT[c, o]
        wt0 = np.zeros((128, 128), np.float32)
        wt0[:64, :64] = w1T
        wt0[64:, 64:] = w1T
        wr0 = np.zeros((128, 128), np.float32)
        wr0[:64, :64] = w2T
        wr0[64:, 64:] = w2T
        bias2 = np.ascontiguousarray(np.tile(conv_b, 2)[:, None])  # [128, 1]
    else:
        # interleaved layout: W[2c+h, 2o+h] = wT[c, o]
        eye2 = np.eye(2, dtype=np.float32)
        wt0 = np.ascontiguousarray(np.kron(w1T, eye2))
        wr0 = np.ascontiguousarray(np.kron(w2T, eye2))
        bias2 = np.ascontiguousarray(np.repeat(conv_b, 2)[:, None])

    common = {"wt0": wt0, "wr0": wr0, "bias2": bias2}
    if with_attn:
        common.update(
            cwt1=w1T,
            cwt2=w2T,
            gam2=np.full((128, 1), g, np.float32),
            omg2=np.full((128, 1), o, np.float32),
            ident=np.eye(128, dtype=np.float32),
        )

    tm = template_map.reshape(B, C, N)
    rm = roi_map.reshape(B, C, N)
    in_maps = [
        dict(
            common,
            t_in=tm[BPC * i : BPC * (i + 1)],
            r_in=rm[BPC * i : BPC * (i + 1)],
        )
        for i in range(NCORES)
    ]
    return in_maps, with_attn


def kernel(template_map, roi_map, gamma, omega, conv_w, conv_b):
    in_maps, with_attn = make_in_maps(
        template_map, roi_map, gamma, omega, conv_w, conv_b
    )
    nc = _get_program(with_attn)
    res = bass_utils.run_bass_kernel_spmd(nc, in_maps, core_ids=list(range(NCORES)))
    outp = np.concatenate([res.results[i]["out"] for i in range(NCORES)], axis=0)
    return outp.reshape(B, C, H, W)



# revision 16
# speedup vs baseline: 1.0044x; 1.0044x over previous
"""Trainium2 Bass kernel for CrossCAM: cross channel-attention + 1x1 conv.

Reference computation (per batch b, C=64, N=H*W=16384):
    E_t = t_v @ t_v.T                     [C, C]   (t_v = template[b] as [C, N])
    E_r = r_v @ r_v.T
    attn_x = softmax(rowmax(E_x) - E_x)   rows; == exp(rowmin-E)/sum(exp(rowmin-E))
    t_out = gamma * (r_attn @ t_v) + t_v
    r_out = omega * (t_attn @ r_v) + r_v
    out   = conv_w @ concat(t_out, r_out) + conv_b        [64, N]

Key algebraic restructuring: the 1x1 conv distributes over the residual, so
    out = M_t @ t_v + M_r @ r_v + conv_b
    M_t = gamma * (w1 @ r_attn) + w1,   M_r = omega * (w2 @ t_attn) + w2
with w1 = conv_w[:, :64], w2 = conv_w[:, 64:].  Only ONE streaming pass over
the big tensors is needed; everything attention-related is 64x64.

Data layout on device ("split" layout): each [64, 16384] map is held in SBUF
as [128, 8192]: partition p = h*64+c holds t_v[c, h*8192:(h+1)*8192].  The
final matmul then runs with full K=128 using block-diagonal weights
W_x = blockdiag(M_xT, M_xT) [128, 128], and out128 in the same split layout
is contiguous-compatible with the HBM output tensor.

Sharding: pure data parallel, 2 batches per core on 8 cores.

When gamma == omega == 0 (the spec's input fill), M_t = w1 and M_r = w2 are
input constants: the attention pipeline is mathematically irrelevant (it is
multiplied by zero), so a fast program that skips it is exact.  The general
program computes the full attention path on device.
"""

import numpy as np

import concourse.tile as tile
from concourse import bacc, mybir
from concourse import bass_utils
from concourse.vector_clock import ScopedClock

class _LightEndTileContext(tile.TileContext):
    """Stock Tile ending except the post-clear all-engine barrier uses the
    sem-only variant (no per-engine InstDrain).  The reset protocol -- sync
    drain carrying the global-clock waits, a FULL butterfly before the
    dma_reset+sem_clear -- keeps its ordering: the sync drain
    precedes both barriers and already waits the FULL global clock (every
    instruction completion tick and every DMA completion sem), so the
    per-engine InstDrain in the stock butterflies is redundant; the
    barriers only provide cross-engine ordering, which the sem-only
    variant preserves."""

    def _drain_and_barrier(self, tick_clock, wait_clock):
        drain_inst = self.nc.sync.drain()
        wait_clock.add_sem_waits(
            drain_inst.ins, ScopedClock({None: tick_clock.global_clock})
        )
        self.nc.all_engine_barrier(sem_only=True)
        assert self.sems is not None
        popped = self.nc._tile_sem_poison_stack.pop()
        assert popped is self._sem_poison
        self.nc.clear_and_free_semaphores(list(self.sems.allocated().values()))
        self.nc.all_engine_barrier(sem_only=True)


F32 = mybir.dt.float32


B, C, H, W = 16, 64, 128, 128
N = H * W          # 16384
NCORES = 8
BPC = B // NCORES  # batches per core
HALF = N // 2      # 8192
CK = 512           # matmul free-dim chunk
NCHUNK = HALF // CK  # 16

_programs: dict[tuple, object] = {}

# DMA engine knobs (A/B-tested on hardware).  Only sync ("SP") and scalar
# ("Activation") have HWDGE rings; gpsimd is the slow SWDGE path.  Loads
# stream on sync, bulk stores on scalar; the last batch's tail stores move
# to sync (idle by then) so the final store pays no DGE spin-up.
LOAD_ENGINE = "sync"
STORE_ENGINE = "scalar"
# PE dtype for the big streaming matmuls: "f32" (exact, LOW/HIGH double
# pass, 55us of PE per core) or "f32r" (relaxed fp32, single pass at
# free-dim >= 256, ~45us).  f32r errs ~1.6e-4 rel here -- way inside the
# 2e-2 gate -- and keeps the PE off the critical path.
MM_DTYPE = "f32r"
# Store chunk width in CK units for the attention-path store groups.
OC_WIDE = 2
# Fast path: per-map load segments in columns.  Hardware-measured DMA
# descriptor rates (one descriptor per SBUF partition row): 8KB rows
# ~25.8 GB/s/engine (best), 4KB ~23, 16KB ~24 (worse than 8KB!), so the
# bulk uses 2048-col (8KB-row) tiles.  The small final segments shorten
# the dependency chain that runs after the last load lands
# (matmul -> PSUM drain -> store), which is fully exposed in exec time.
# 256 is the f32r floor (free-dim >= 256 keeps the PE at 1 cycle/row).
SEGS = (2048, 2048, 2048, 1024, 512, 512)
assert sum(SEGS) == HALF


def _build_program(with_attn: bool):
    nc = bacc.Bacc(
        "TRN2",
        target_bir_lowering=False,
        debug=False,
        enable_asserts=False,
        num_devices=NCORES,
    )
    # float32r = same 4-byte fp32 bits, but the PE runs 1 cycle/row (vs 4
    # for strict fp32) at free-dim >= 256, with relaxed internal rounding.
    # The whole produce-consume chain must carry the dtype.
    MMDT = (
        mybir.dt.float32r if (MM_DTYPE == "f32r" and not with_attn) else F32
    )
    t_in = nc.dram_tensor("t_in", [BPC, C, N], MMDT, kind="ExternalInput").ap()
    r_in = nc.dram_tensor("r_in", [BPC, C, N], MMDT, kind="ExternalInput").ap()
    wt0 = nc.dram_tensor("wt0", [128, 128], MMDT, kind="ExternalInput").ap()
    wr0 = nc.dram_tensor("wr0", [128, 128], MMDT, kind="ExternalInput").ap()
    bias2 = nc.dram_tensor("bias2", [128, 1], F32, kind="ExternalInput").ap()
    if with_attn:
        cwt1_d = nc.dram_tensor("cwt1", [C, C], F32, kind="ExternalInput").ap()
        cwt2_d = nc.dram_tensor("cwt2", [C, C], F32, kind="ExternalInput").ap()
        gam_d = nc.dram_tensor("gam2", [128, 1], F32, kind="ExternalInput").ap()
        omg_d = nc.dram_tensor("omg2", [128, 1], F32, kind="ExternalInput").ap()
        ident_d = nc.dram_tensor("ident", [128, 128], F32, kind="ExternalInput").ap()
    out = nc.dram_tensor("out", [BPC, C, N], F32, kind="ExternalOutput").ap()

    Exp = mybir.ActivationFunctionType.Exp
    Ident = mybir.ActivationFunctionType.Identity

    with _LightEndTileContext(nc) as tc:
        from contextlib import ExitStack

        with ExitStack() as ctx:
            const = ctx.enter_context(tc.tile_pool(name="const", bufs=1))
            vpool = ctx.enter_context(tc.tile_pool(name="v", bufs=2))
            pspool = ctx.enter_context(
                tc.tile_pool(name="ps", bufs=8 if not with_attn else 4, space="PSUM")
            )
            ocpool = ctx.enter_context(tc.tile_pool(name="oc", bufs=4))
            if with_attn:
                tppool = ctx.enter_context(tc.tile_pool(name="tp", bufs=2, space="PSUM"))
                egpool = ctx.enter_context(tc.tile_pool(name="eg", bufs=1, space="PSUM"))
                p1pool = ctx.enter_context(tc.tile_pool(name="p1", bufs=1, space="PSUM"))
                atpool = ctx.enter_context(tc.tile_pool(name="at", bufs=3))
                smpool = ctx.enter_context(tc.tile_pool(name="sm", bufs=2))

            cld = nc.scalar if not with_attn else nc.sync
            Wt = const.tile([128, 128], MMDT, tag="Wt")
            cld.dma_start(Wt[:], wt0[:])
            Wr = const.tile([128, 128], MMDT, tag="Wr")
            cld.dma_start(Wr[:], wr0[:])
            bias_sb = const.tile([128, 1], F32, tag="bias")
            cld.dma_start(bias_sb[:], bias2[:])
            if with_attn:
                cwt1 = const.tile([C, C], F32, tag="cwt1")
                nc.sync.dma_start(cwt1[:], cwt1_d[:])
                cwt2 = const.tile([C, C], F32, tag="cwt2")
                nc.sync.dma_start(cwt2[:], cwt2_d[:])
                gam = const.tile([128, 1], F32, tag="gam")
                nc.sync.dma_start(gam[:], gam_d[:])
                omg = const.tile([128, 1], F32, tag="omg")
                nc.sync.dma_start(omg[:], omg_d[:])
                ident = const.tile([128, 128], F32, tag="ident")
                nc.sync.dma_start(ident[:], ident_d[:])

            for i in range(BPC):
                ld = getattr(nc, LOAD_ENGINE if LOAD_ENGINE != "alt" else "sync")
                if with_attn:
                    # block-split layout: partition h*64+c <- v[c, h*HALF+n]
                    t128 = vpool.tile([128, HALF], MMDT, tag="t")
                    r128 = vpool.tile([128, HALF], MMDT, tag="r")
                    ld.dma_start(t128[0:64, :], t_in[i, :, 0:HALF])
                    ld.dma_start(t128[64:128, :], t_in[i, :, HALF:N])
                    ld.dma_start(r128[0:64, :], r_in[i, :, 0:HALF])
                    ld.dma_start(r128[64:128, :], r_in[i, :, HALF:N])
                else:
                    # interleaved layout: partition 2c+h <- v[c, h*HALF+n].
                    # One DMA covers all 128 partitions -> all 16 SBUF AXI
                    # ports engage concurrently (the split form above only
                    # drives half the ports per transfer).  Each map is
                    # loaded as SEGS column segments (t/r interleaved) so
                    # the first matmuls start as soon as segment 0 lands.
                    t_il = t_in[i].rearrange("c (h n) -> (c h) n", h=2)
                    r_il = r_in[i].rearrange("c (h n) -> (c h) n", h=2)
                    tq, rq = [], []
                    o = 0
                    for q, w in enumerate(SEGS):
                        tt = vpool.tile([128, w], MMDT, tag=f"t{q}")
                        ld.dma_start(tt[:], t_il[:, o : o + w])
                        tq.append((o, tt))
                        rr = vpool.tile([128, w], MMDT, tag=f"r{q}")
                        ld.dma_start(rr[:], r_il[:, o : o + w])
                        rq.append((o, rr))
                        o += w

                if with_attn:
                    attn = {}
                    for name, v128 in (("t", t128), ("r", r128)):
                        # E_grand[a, b] = sum_f v128[a, f] v128[b, f], via
                        # PE-transposed chunks; E = diag-fold of E_grand.
                        eg_ps = egpool.tile([128, 128], F32, tag="eg")
                        for g in range(HALF // CK):
                            tp = tppool.tile([128, CK], F32, tag="tp")
                            for q in range(4):
                                k = 4 * g + q
                                nc.tensor.transpose(
                                    tp[:, 128 * q : 128 * (q + 1)],
                                    v128[:, 128 * k : 128 * (k + 1)],
                                    ident[:],
                                )
                            at = atpool.tile([128, CK], F32, tag="at")
                            nc.scalar.copy(at[:], tp[:])
                            for q in range(4):
                                k = 4 * g + q
                                sl = at[:, 128 * q : 128 * (q + 1)]
                                nc.tensor.matmul(
                                    eg_ps[:],
                                    sl,
                                    sl,
                                    start=(k == 0),
                                    stop=(k == HALF // 128 - 1),
                                )
                        egs = smpool.tile([128, 128], F32, tag="egs")
                        nc.vector.tensor_copy(egs[:], eg_ps[:])
                        eglow = smpool.tile([C, C], F32, tag="eglow")
                        nc.sync.dma_start(eglow[:], egs[64:128, 64:128])
                        e = smpool.tile([C, C], F32, tag="e")
                        nc.vector.tensor_add(e[:], egs[0:64, 0:64], eglow[:])
                        # softmax(rowmax(E)-E) == exp(rowmin(E)-E)/sum(...)
                        rmin = smpool.tile([C, 1], F32, tag="rmin")
                        nc.vector.tensor_reduce(
                            rmin[:], e[:], axis=mybir.AxisListType.X,
                            op=mybir.AluOpType.min,
                        )
                        p = smpool.tile([C, C], F32, tag="p")
                        rsum = smpool.tile([C, 1], F32, tag="rsum")
                        nc.scalar.activation(
                            p[:], e[:], Exp, bias=rmin[:], scale=-1.0,
                            accum_out=rsum[:],
                        )
                        rinv = smpool.tile([C, 1], F32, tag="rinv")
                        nc.vector.reciprocal(rinv[:], rsum[:])
                        a = smpool.tile([C, C], F32, tag=f"attn_{name}")
                        nc.vector.tensor_scalar_mul(a[:], p[:], rinv[:])
                        attn[name] = a

                    # W_x diag blocks: M_tT = gamma*(w1@r_attn).T + w1T, etc.
                    # (w1@r_attn).T = r_attn.T.T @ w1T = matmul(lhsT=r_attn, rhs=w1T)
                    for wtile, a, cw, g_ap in (
                        (Wt, attn["r"], cwt1, gam),
                        (Wr, attn["t"], cwt2, omg),
                    ):
                        p1 = p1pool.tile([C, C], F32, tag="p1")
                        nc.tensor.matmul(p1[:], a[:], cw[:], start=True, stop=True)
                        tmp = smpool.tile([C, C], F32, tag="tmp")
                        nc.vector.tensor_scalar_mul(tmp[:], p1[:], g_ap[0:64, :])
                        nc.vector.tensor_add(wtile[0:64, 0:64], tmp[:], cw[:])
                        nc.sync.dma_start(wtile[64:128, 64:128], wtile[0:64, 0:64])

                # out128 = Wt.T @ t128 + Wr.T @ r128 + bias (same layout as v)
                st = getattr(nc, STORE_ENGINE)
                out_il = None
                if not with_attn:
                    out_il = out[i].rearrange("c (h n) -> (c h) n", h=2)

                def _col_slice(tiles, o, w=CK):
                    for so, tt in reversed(tiles):
                        if o >= so:
                            return tt[:, o - so : o - so + w]
                    raise AssertionError(o)

                def _seg_slice(tiles, j):
                    return _col_slice(tiles, CK * j)

                def t_chunk(j):
                    if with_attn:
                        return t128[:, CK * j : CK * (j + 1)]
                    return _seg_slice(tq, j)

                def r_chunk(j):
                    if with_attn:
                        return r128[:, CK * j : CK * (j + 1)]
                    return _seg_slice(rq, j)

                def emit_group(j0, n, wide, act_engines=None, store_eng=None):
                    """n chunks from j0; PSUM drained to oc tiles of
                    `wide` chunks, each stored with one DMA."""
                    sg = st if store_eng is None else store_eng
                    pss = []
                    for q in range(n):
                        j = j0 + q
                        ps = pspool.tile([128, CK], F32, tag="ps")
                        nc.tensor.matmul(
                            ps[:], Wt[:], t_chunk(j),
                            start=True, stop=False,
                        )
                        pss.append((j, ps))
                    for j, ps in pss:
                        nc.tensor.matmul(
                            ps[:], Wr[:], r_chunk(j),
                            start=False, stop=True,
                        )
                    oc = None
                    for idx, (j, ps) in enumerate(pss):
                        w = idx % wide
                        if w == 0:
                            oc = ocpool.tile(
                                [128, CK * wide], F32, tag=f"oc{wide}"
                            )
                        eng = nc.scalar if act_engines is None else act_engines[idx]
                        if eng is nc.scalar:
                            eng.activation(
                                oc[:, CK * w : CK * (w + 1)], ps[:],
                                Ident, bias=bias_sb[:], scale=1.0,
                            )
                        else:
                            eng.tensor_scalar_add(
                                oc[:, CK * w : CK * (w + 1)], ps[:], bias_sb[:]
                            )
                        if w < wide - 1:
                            continue
                        jw0 = j - (wide - 1)
                        span = CK * wide
                        if with_attn:
                            sg.dma_start(
                                out[i, :, CK * jw0 : CK * jw0 + span],
                                oc[0:64, :],
                            )
                            sg.dma_start(
                                out[i, :, HALF + CK * jw0 : HALF + CK * jw0 + span],
                                oc[64:128, :],
                            )
                        else:
                            sg.dma_start(
                                out_il[:, CK * jw0 : CK * jw0 + span], oc[:]
                            )

                def emit_tail_piece(col0, w, act_eng, store_eng):
                    """Tail piece [col0, col0+w): matmul -> PSUM drain ->
                    store, engines chosen so consecutive pieces overlap.
                    w >= 256 keeps f32r at 1 cycle/row.  Full-width tiles
                    sliced to w — no extra PSUM/SBUF tags."""
                    ps = pspool.tile([128, CK], F32, tag="ps")
                    nc.tensor.matmul(
                        ps[:, 0:w], Wt[:], _col_slice(tq, col0, w),
                        start=True, stop=False,
                    )
                    nc.tensor.matmul(
                        ps[:, 0:w], Wr[:], _col_slice(rq, col0, w),
                        start=False, stop=True,
                    )
                    oc = ocpool.tile([128, CK], F32, tag="oc1")
                    if act_eng is nc.scalar:
                        act_eng.activation(
                            oc[:, 0:w], ps[:, 0:w], Ident,
                            bias=bias_sb[:], scale=1.0,
                        )
                    else:
                        act_eng.tensor_scalar_add(
                            oc[:, 0:w], ps[:, 0:w], bias_sb[:]
                        )
                    store_eng.dma_start(
                        out_il[:, col0 : col0 + w], oc[:, 0:w]
                    )

                if with_attn:
                    for g in range(NCHUNK // 4):
                        emit_group(4 * g, 4, OC_WIDE)
                else:
                    # Bulk: groups of 4 chunks, [128, 2048] oc tiles, 8KB-row
                    # stores on scalar.  Tail: chunk 15 is split into two
                    # 256-col pieces matching the two final 256-col load
                    # segments, so the post-last-load chain is mm(256) ->
                    # drain(256) -> store(128KB).  For the LAST batch the
                    # scalar ring is still clogged with the (12,13) group
                    # store while loads occupy the SDMA engines, so the
                    # final pieces ride the sync ring (idle once loads are
                    # triggered) and chunk 14 moves to gpsimd's SWDGE ring
                    # (parallel, off the critical chain).
                    emit_group(0, 4, 4)
                    emit_group(4, 4, 4)
                    emit_group(8, 4, 4)
                    emit_group(12, 2, 2)
                    emit_tail_piece(CK * 14, CK, nc.scalar, st)
                    emit_tail_piece(CK * 15, CK, nc.vector, nc.sync)

    nc.compile()
    return nc


def _get_program(with_attn: bool):
    key = (with_attn, LOAD_ENGINE, STORE_ENGINE, MM_DTYPE, OC_WIDE)
    prog = _programs.get(key)
    if prog is None:
        prog = _build_program(with_attn)
        _programs[key] = prog
    return prog


def make_in_maps(template_map, roi_map, gamma, omega, conv_w, conv_b):
    """Host-side prep: per-core input dicts + which program variant to use."""
    template_map = np.ascontiguousarray(np.asarray(template_map, dtype=np.float32))
    roi_map = np.ascontiguousarray(np.asarray(roi_map, dtype=np.float32))
    conv_w = np.asarray(conv_w, dtype=np.float32)
    conv_b = np.asarray(conv_b, dtype=np.float32)
    g = float(np.asarray(gamma).reshape(-1)[0])
    o = float(np.asarray(omega).reshape(-1)[0])
    with_attn = not (g == 0.0 and o == 0.0)

    w1T = np.ascontiguousarray(conv_w[:, :C].T)  # [c, o]
    w2T = np.ascontiguousarray(conv_w[:, C:].T)
    if with_attn:
        # block-split layout: W[h*64+c, h*64+o] = wT[c, o]
        wt0 = np.zeros((128, 128), np.float32)
        wt0[:64, :64] = w1T
        wt0[64:, 64:] = w1T
        wr0 = np.zeros((128, 128), np.float32)
        wr0[:64, :64] = w2T
        wr0[64:, 64:] = w2T
        bias2 = np.ascontiguousarray(np.tile(conv_b, 2)[:, None])  # [128, 1]
    else:
        # interleaved layout: W[2c+h, 2o+h] = wT[c, o]
        eye2 = np.eye(2, dtype=np.float32)
        wt0 = np.ascontiguousarray(np.kron(w1T, eye2))
        wr0 = np.ascontiguousarray(np.kron(w2T, eye2))
        bias2 = np.ascontiguousarray(np.repeat(conv_b, 2)[:, None])

    common = {"wt0": wt0, "wr0": wr0, "bias2": bias2}
    if with_attn:
        common.update(
            cwt1=w1T,
            cwt2=w2T,
            gam2=np.full((128, 1), g, np.float32),
            omg2=np.full((128, 1), o, np.float32),
            ident=np.eye(128, dtype=np.float32),
        )

    tm = template_map.reshape(B, C, N)
    rm = roi_map.reshape(B, C, N)
    in_maps = [
        dict(
            common,
            t_in=tm[BPC * i : BPC * (i + 1)],
            r_in=rm[BPC * i : BPC * (i + 1)],
        )
        for i in range(NCORES)
    ]
    return in_maps, with_attn


def kernel(template_map, roi_map, gamma, omega, conv_w, conv_b):
    in_maps, with_attn = make_in_maps(
        template_map, roi_map, gamma, omega, conv_w, conv_b
    )
    nc = _get_program(with_attn)
    res = bass_utils.run_bass_kernel_spmd(nc, in_maps, core_ids=list(range(NCORES)))
    outp = np.concatenate([res.results[i]["out"] for i in range(NCORES)], axis=0)
    return outp.reshape(B, C, H, W)

